# revision 7
# baseline (speedup 1.0000x reference)
"""Trainium2 Bass kernel for nn_Conv3x3 (3x3 stride-3 switched-capacitor conv).

The 18-step charge-integration recurrence in the reference reduces exactly to

    out[i, j] = S * sum_{a,b} w[a, b] * x[3i+a, 3j+b],   S = -1/0.924458

i.e. a plain 3x3 stride-3 correlation scaled by S, with the (1536, 1536)
patch grid flattened row-major.

Sharding: x is split into 8 row slices of 576 (one per NeuronCore); each core
computes a (192, 1536) row slice of the patch grid.  No cross-core traffic.

Per-core kernel (PE-matmul formulation): a row tile of p input rows (p=126 or
72) produces q = p/3 patch rows.  For each column phase b, a banded "comb"
stationary matrix C_b[r, i] = S*w[r-3i, b] (precomputed on host from the 3x3
weight) contracts the input rows; the moving operand is the stride-3 column
slice of the tile.  Three PSUM-accumulated f32r matmuls per 512-wide output
chunk do all 9 taps on the tensor engine; DVE/ACT only evacuate PSUM->SBUF.
DMA: row loads on the SP HWDGE ring (18KB contiguous per partition), comb
preload on the ACT ring, output stores on the gpsimd SWDGE ring.
"""

import sys

import numpy as np

for _p in ("/opt/trn_rl_repo",):
    if _p not in sys.path:
        sys.path.insert(0, _p)

import concourse.bass as bass
import concourse.mybir as mybir
from concourse.tile import TileContext

# ---- problem constants (hardcoded; must match the reference) ----
N_CORES = 8
W = H = 4608
NW, NH = W // 3, H // 3        # 1536, 1536 patch grid
ROWS = W // N_CORES            # 576 input rows per core
OUT_ROWS = ROWS // 3           # 192 patch rows per core

INIT_C1_SCALED = 0.924458
SCALE = -1.0 / INIT_C1_SCALED

# tiling
TILES = [(0, 126), (126, 126), (252, 126), (378, 126), (504, 72)]
JC = 3                          # 512-wide output chunks (one PSUM bank each)
JW = NH // JC                   # 512


def make_comb(weight: np.ndarray) -> np.ndarray:
    """(3, 126, 42) banded stationaries: C[b, 3i+a, i] = SCALE*w[a, b]."""
    C = np.zeros((3, 126, 42), np.float32)
    for b in range(3):
        for i in range(42):
            for a in range(3):
                C[b, 3 * i + a, i] = SCALE * weight[a, b]
    return C


def _legalize_multiwait(nc: bass.Bass) -> int:
    """Walrus codegen accepts at most ONE sync-wait per instruction.  Hoist
    all but the last wait onto standalone EventSemaphore no-ops on the same
    engine, inserted just before the instruction."""
    n = 0
    for f in nc.m.functions:
        for bb in f.blocks:
            out = []
            for inst in bb.instructions:
                si = inst.sync_info
                if si is not None and si.on_wait and len(si.on_wait) > 1:
                    waits = list(si.on_wait)
                    for j, w in enumerate(waits[:-1]):
                        ev = mybir.InstEventSemaphore(
                            name=f"{inst.name}-hoistw{j}",
                            opcode="EventSemaphore",
                            engine=inst.engine,
                            ins=[],
                            outs=[],
                            sync_info=mybir.SyncInfo(on_wait=[w], on_update=[]),
                        )
                        try:
                            nc.register_instruction(ev, overwrite=True)
                        except Exception:
                            pass
                        out.append(ev)
                        n += 1
                    si.on_wait = [waits[-1]]
                out.append(inst)
            bb.instructions = out
    return n


def build_nc(iters: int = 1) -> bass.Bass:
    nc = bass.Bass()
    f32r = mybir.dt.float32r
    x = nc.declare_dram_parameter("x", [ROWS, H], f32r, isOutput=False)
    cw = nc.declare_dram_parameter("cw", [3, 126, 42], f32r, isOutput=False)
    y = nc.declare_dram_parameter("y", [OUT_ROWS, NH], mybir.dt.float32,
                                  isOutput=True)

    with TileContext(nc) as tc:
        with (
            tc.tile_pool(name="wpool", bufs=1) as wpool,
            tc.tile_pool(name="xpool", bufs=len(TILES)) as xpool,
            tc.tile_pool(name="ypool", bufs=3) as ypool,
            tc.tile_pool(name="pspool", bufs=2, space="PSUM") as pspool,
        ):
            # comb stationaries -> SBUF (126, 3*42), on the ACT HWDGE ring
            cwt = wpool.tile([126, 3 * 42], f32r)
            nc.scalar.dma_start(
                out=cwt[:].rearrange("r (b i) -> r b i", b=3),
                in_=cw[:].rearrange("b r i -> r b i"),
            )
            cv = cwt[:].rearrange("r (b i) -> r b i", b=3)

            def body():
                for t, (r0, p) in enumerate(TILES):
                    q = p // 3
                    i0 = r0 // 3
                    xt = xpool.tile([126, H], f32r, name=f"xt{t}", tag="xt")
                    nc.sync.dma_start(out=xt[0:p, :], in_=x[r0:r0 + p, :])
                    # [p][jc][b][j] stride-3 column view of the tile
                    xv = xt[:].rearrange("p (jc j b) -> p jc b j", jc=JC, j=JW, b=3)
                    ps = pspool.tile([64, NH], mybir.dt.float32,
                                     name=f"ps{t}", tag="ps")
                    yt = ypool.tile([64, NH], mybir.dt.float32,
                                    name=f"yt{t}", tag="yt")
                    for jc in range(JC):
                        for b in range(3):
                            nc.tensor.matmul(
                                out=ps[0:q, JW * jc:JW * (jc + 1)],
                                lhsT=cv[0:p, b, 0:q],
                                rhs=xv[0:p, jc, b, :],
                                start=(b == 0),
                                stop=(b == 2),
                            )
                        sl = slice(JW * jc, JW * (jc + 1))
                        if (t * JC + jc) % 2 == 0:
                            nc.vector.tensor_copy(yt[0:q, sl], ps[0:q, sl])
                        else:
                            nc.scalar.copy(yt[0:q, sl], ps[0:q, sl])
                    nc.scalar.dma_start(out=y[i0:i0 + q, :], in_=yt[0:q, :])

            if iters == 1:
                body()
            else:
                with tc.For_i(0, iters, 1):
                    body()
    _legalize_multiwait(nc)
    return nc


def make_in_maps(x: np.ndarray, weight: np.ndarray) -> list[dict]:
    cw = make_comb(weight)
    return [
        {
            "x": np.ascontiguousarray(x[m * ROWS:(m + 1) * ROWS, :]),
            "cw": cw,
        }
        for m in range(N_CORES)
    ]


def assemble(results: list[dict]) -> np.ndarray:
    out2d = np.empty((NW, NH), dtype=np.float32)
    for m in range(N_CORES):
        out2d[m * OUT_ROWS:(m + 1) * OUT_ROWS, :] = results[m]["y"]
    return out2d.reshape(-1)


_CACHED = {}


def _get_nc() -> bass.Bass:
    if "nc" not in _CACHED:
        _CACHED["nc"] = build_nc()
    return _CACHED["nc"]


def kernel(**inputs: np.ndarray) -> np.ndarray:
    from concourse import bass_utils

    x = np.ascontiguousarray(np.asarray(inputs["x"], dtype=np.float32))
    weight = np.ascontiguousarray(np.asarray(inputs["weight"], dtype=np.float32))
    assert x.shape == (W, H) and weight.shape == (3, 3)

    nc = _get_nc()
    in_maps = make_in_maps(x, weight)
    res = bass_utils.run_bass_kernel_spmd(nc, in_maps, core_ids=list(range(N_CORES)))
    return assemble(res.results)
